# revision 1
# baseline (speedup 1.0000x reference)
"""Trainium2 Bass kernel for CombinedModel cosine-sim attention pooling.

Reference computation (per batch sample b):
    f1  = features[b] @ W + b_vec                     # [N, D]
    t1  = text[1]                                     # [M, D]
    fn  = f1 / ||f1||_row ; tn = t1 / ||t1||_row
    sim = fn @ tn.T                                   # [N, M]
    w   = exp(sim) / sum_n exp(sim)                   # column softmax-ish over N
    fm  = w.T @ features[b]                           # [M, D]
    out = concat([fm, t1], -1)                        # [M, 2D]

Sharding: data-parallel over batch B=8 across the 8 NeuronCores (one sample
per core).  All matmuls run as float32r (TF32-like) at full PE rate; the
column-sum normalization commutes with the N-accumulation so everything is a
single pass over N:  fm = (sum_n e[n,m] f[n,d]) / s[m], s[m] = sum_n e[n,m].
"""

from contextlib import ExitStack

import numpy as np

import concourse.bass as bass
import concourse.mybir as mybir
import concourse.tile as tile
from concourse import bacc
from concourse.bass_utils import run_bass_kernel_spmd
from concourse.masks import make_identity

B, N, M, D = 8, 4096, 2048, 512
P = 128
NB = N // P          # 32 n-blocks
NCH = N // 512       # 8 n-chunks (4 blocks each)
EG = D // P          # 4 e-groups (output dim of linear layer)
MCH = M // 512       # 4 m-chunks
F32 = mybir.dt.float32
F32R = mybir.dt.float32r
AF = mybir.ActivationFunctionType
AX = mybir.AxisListType

_NC_CACHE = {}


def build_nc():
    nc = bacc.Bacc("TRN2")

    features_h = nc.dram_tensor("features", [N, D], F32, kind="ExternalInput")
    t1_h = nc.dram_tensor("t1", [M, D], F32, kind="ExternalInput")
    w_h = nc.dram_tensor("W", [D, D], F32, kind="ExternalInput")
    b_h = nc.dram_tensor("b", [D], F32, kind="ExternalInput")
    out_h = nc.dram_tensor("out", [M, 2 * D], F32, kind="ExternalOutput")

    f_re = features_h.ap().rearrange("(nb p) d -> p nb d", p=P)      # [128,32,512]
    t1_re = t1_h.ap().rearrange("(mb p) d -> p mb d", p=P)           # [128,16,512]
    w_re = w_h.ap().rearrange("(dg p) e -> p dg e", p=P)             # [128,4,512]
    out_re = out_h.ap().rearrange("(mb p) c -> p mb c", p=P)         # [128,16,1024]

    with tile.TileContext(nc) as tc, ExitStack() as top:
        singles = top.enter_context(tc.tile_pool(name="singles", bufs=1))
        f1t_pool = top.enter_context(tc.tile_pool(name="f1t", bufs=1))
        tnt_pool = top.enter_context(tc.tile_pool(name="tnt", bufs=1))
        small = top.enter_context(tc.tile_pool(name="small", bufs=4))
        pg = top.enter_context(tc.tile_pool(name="pg", bufs=2, space="PSUM"))
        dram = top.enter_context(tc.tile_pool(name="dram", bufs=2, space="DRAM"))

        # --- constants ---
        ident = singles.tile([P, P], F32)
        make_identity(nc, ident)
        ident_r = singles.tile([P, P], F32R)
        nc.scalar.copy(out=ident_r, in_=ident)
        w_sb = singles.tile([P, EG, D], F32R)          # W[d, e], d = 128*dg + p
        for dg in range(EG):
            nc.gpsimd.dma_start(
                out=w_sb[:, dg, :], in_=w_re[:, dg, :].bitcast(F32R)
            )
        bt = singles.tile([P, EG], F32)               # b[e], e = 128*g + p
        nc.gpsimd.dma_start(out=bt, in_=b_h.ap().rearrange("(g p) -> p g", p=P))
        ones_f32 = singles.tile([P, 1], F32)
        nc.vector.memset(ones_f32, 1.0)
        ones_col = singles.tile([P, 1], F32R)
        nc.scalar.copy(out=ones_col, in_=ones_f32)
        ss_c = [singles.tile([P, 4], F32, tag=f"ss{c}", name=f"ss{c}") for c in range(NCH)]
        rf_c = [singles.tile([P, 4], F32, tag=f"rf{c}", name=f"rf{c}") for c in range(NCH)]
        f1t = [f1t_pool.tile([P, EG, 512], F32R, tag=f"f1t{c}", name=f"f1t{c}") for c in range(NCH)]
        tnt = [tnt_pool.tile([P, EG, 512], F32R, tag=f"tnt{s}", name=f"tnt{s}") for s in range(MCH)]

        with ExitStack() as ph:
            featp = ph.enter_context(tc.tile_pool(name="featp", bufs=2))
            ftp = ph.enter_context(tc.tile_pool(name="ftp", bufs=2))
            sqp = ph.enter_context(tc.tile_pool(name="sqp", bufs=2))
            t1p = ph.enter_context(tc.tile_pool(name="t1p", bufs=2))
            ptr = ph.enter_context(tc.tile_pool(name="ptr", bufs=2, space="PSUM"))
            pf1t = ph.enter_context(tc.tile_pool(name="pf1t", bufs=2, space="PSUM"))

            tns_tiles = {}

            def emit_t1_strip_a(s):
                t1s = t1p.tile([P, 4, 512], F32, name="t1s")
                nc.gpsimd.dma_start(out=t1s, in_=t1_re[:, 4 * s : 4 * s + 4, :])
                nc.gpsimd.dma_start(
                    out=out_re[:, 4 * s : 4 * s + 4, D : 2 * D], in_=t1s
                )
                sq2 = sqp.tile([P, 4, 512], F32, tag="sq2", name="sq2")
                nc.vector.tensor_mul(sq2, t1s, t1s)
                rt = small.tile([P, 4], F32, tag="rt", name="rt")
                nc.vector.reduce_sum(out=rt, in_=sq2, axis=AX.X)
                nc.scalar.sqrt(out=rt, in_=rt)
                nc.vector.reciprocal(out=rt, in_=rt)
                tns = t1p.tile([P, 4, 512], F32R, tag="tns", name="tns")
                for j in range(4):
                    nc.vector.tensor_scalar_mul(
                        out=tns[:, j, :], in0=t1s[:, j, :], scalar1=rt[:, j : j + 1]
                    )
                tns_tiles[s] = tns

            def emit_t1_strip_b(s):
                tns = tns_tiles.pop(s)
                for dg in range(EG):
                    ptt = ptr.tile([P, 512], F32R, tag="ptt", name="ptt", bufs=1)
                    for j in range(4):
                        nc.tensor.transpose(
                            ptt[:, j * P : (j + 1) * P],
                            tns[:, j, dg * P : (dg + 1) * P],
                            ident_r,
                        )
                    nc.vector.tensor_copy(out=tnt[s][:, dg, :], in_=ptt)

            # --- phase 1: features -> fT strips -> f1T (+bias) and row norms ---
            featc_tiles = {}

            def load_featc(cc):
                featc = featp.tile([P, 4, 512], F32R, name="featc")
                for j in range(4):
                    nc.sync.dma_start(
                        out=featc[:, j, :], in_=f_re[:, 4 * cc + j, :].bitcast(F32R)
                    )
                featc_tiles[cc] = featc

            load_featc(0)
            for c in range(NCH):
                featc = featc_tiles.pop(c)
                if c + 1 < NCH:
                    load_featc(c + 1)
                ftc = ftp.tile([P, EG, 512], F32R)     # features^T[d, n-chunk]
                for dg in range(EG):
                    pt = ptr.tile([P, 512], F32R, bufs=2, name="pt")
                    for j in range(4):
                        nc.tensor.transpose(
                            pt[:, j * P : (j + 1) * P],
                            featc[:, j, dg * P : (dg + 1) * P],
                            ident_r,
                        )
                    nc.vector.tensor_copy(out=ftc[:, dg, :], in_=pt)
                # f1T[e, n] for this n-chunk, e = 128*g + p
                for g in range(EG):
                    pf = pf1t.tile([P, 512], F32)
                    for dg in range(EG):
                        nc.tensor.matmul(
                            pf,
                            w_sb[:, dg, g * P : (g + 1) * P],
                            ftc[:, dg, :],
                            start=(dg == 0),
                            stop=(dg == EG - 1),
                        )
                    nc.scalar.activation(
                        out=f1t[c][:, g, :],
                        in_=pf,
                        func=AF.Identity,
                        bias=bt[:, g : g + 1],
                    )
                # row sumsq via Gram diagonal: diag(f1t_blk.T @ f1t_blk),
                # pipelined one chunk behind f1T to hide the ACT-copy latency
                def emit_gram(cc):
                    for j in range(4):
                        gram = ptr.tile([P, P], F32, tag="gram", bufs=1, name="gram")
                        for g in range(EG):
                            blk = f1t[cc][:, g, j * P : (j + 1) * P]
                            nc.tensor.matmul(
                                gram, blk, blk, start=(g == 0), stop=(g == EG - 1)
                            )
                        gd = sqp.tile([P, P], F32, tag="gd", name="gd")
                        nc.vector.tensor_mul(gd, gram, ident)
                        nc.vector.reduce_sum(
                            out=ss_c[cc][:, j : j + 1], in_=gd, axis=AX.X
                        )
                    nc.scalar.sqrt(out=rf_c[cc], in_=ss_c[cc])
                    nc.vector.reciprocal(out=rf_c[cc], in_=rf_c[cc])

                if c > 0:
                    emit_gram(c - 1)
                if c == NCH - 1:
                    emit_gram(c)
                if 1 <= c <= MCH:
                    emit_t1_strip_a(c - 1)
                if 2 <= c <= MCH + 1:
                    emit_t1_strip_b(c - 2)


        # --- phase 3: main loop over m-chunks ---
        with ExitStack() as mn:
            featm = mn.enter_context(tc.tile_pool(name="featm", bufs=6))
            ep = mn.enter_context(tc.tile_pool(name="ep", bufs=3))
            sap = mn.enter_context(tc.tile_pool(name="sap", bufs=2))
            outp = mn.enter_context(tc.tile_pool(name="outp", bufs=3))
            pfm = mn.enter_context(tc.tile_pool(name="pfm", bufs=1, space="PSUM"))
            psm = mn.enter_context(tc.tile_pool(name="psm", bufs=2, space="PSUM"))

            for mc in range(MCH):
                fm_ps = [pfm.tile([P, 512], F32, tag=f"fm{j}", name=f"fm{j}") for j in range(4)]
                sacc = sap.tile([P, 512], F32R)
                prev = None  # (et, fnb) of iteration nb-1

                def emit_fm(nb, et, fnb):
                    for j in range(4):
                        nc.tensor.matmul(
                            fm_ps[j],
                            et[:, j * P : (j + 1) * P],
                            fnb,
                            start=(nb == 0),
                            stop=(nb == NB - 1),
                        )

                for nb in range(NB):
                    fnb = featm.tile([P, 512], F32R)
                    nc.sync.dma_start(out=fnb, in_=f_re[:, nb, :].bitcast(F32R))
                    gp = pg.tile([P, 512], F32)
                    c, jj = nb // 4, nb % 4
                    for g in range(EG):
                        nc.tensor.matmul(
                            gp,
                            f1t[c][:, g, jj * P : (jj + 1) * P],
                            tnt[mc][:, g, :],
                            start=(g == 0),
                            stop=(g == EG - 1),
                        )
                    et = ep.tile([P, 512], F32R)
                    nc.scalar.activation(
                        out=et, in_=gp, func=AF.Exp, scale=rf_c[c][:, jj : jj + 1]
                    )
                    if nb == 0:
                        nc.vector.tensor_copy(out=sacc, in_=et)
                    else:
                        nc.vector.tensor_add(
                            sacc, sacc.bitcast(F32), et.bitcast(F32)
                        )
                    if prev is not None:
                        emit_fm(nb - 1, *prev)
                    prev = (et, fnb)
                emit_fm(NB - 1, *prev)
                # s[m] = column sums; rs = 1/s gathered to [m-part, 1] layout
                fm_sb = outp.tile([P, 4, 512], F32, tag="fmsb", name="fmsb")
                if mc < MCH - 1:
                    for j in range(4):
                        nc.scalar.copy(out=fm_sb[:, j, :], in_=fm_ps[j])
                sp = psm.tile([1, 512], F32, bufs=1)
                nc.tensor.matmul(sp, ones_col, sacc)
                s_sb = small.tile([1, 512], F32, tag="s_sb")
                nc.scalar.copy(out=s_sb, in_=sp)
                rs = small.tile([P, 4], F32, tag="rs")
                for j in range(4):
                    pst = psm.tile([P, 1], F32, tag="pst", name="pst", bufs=1)
                    nc.tensor.transpose(
                        pst, s_sb[0:1, j * P : (j + 1) * P], ident[0:1, 0:1]
                    )
                    nc.vector.tensor_copy(out=rs[:, j : j + 1], in_=pst)
                nc.vector.reciprocal(out=rs, in_=rs)
                if mc < MCH - 1:
                    for j in range(4):
                        nc.vector.tensor_scalar_mul(
                            out=fm_sb[:, j, :],
                            in0=fm_sb[:, j, :],
                            scalar1=rs[:, j : j + 1],
                        )
                    nc.sync.dma_start(
                        out=out_re[:, 4 * mc : 4 * mc + 4, 0:D], in_=fm_sb
                    )
                else:
                    # last m-chunk: no next chunk to stall, so scale straight
                    # from PSUM and stream per-j DMAs for the shortest tail
                    for j in range(4):
                        nc.scalar.activation(
                            out=fm_sb[:, j, :],
                            in_=fm_ps[j],
                            func=AF.Copy,
                            scale=rs[:, j : j + 1],
                        )
                        nc.sync.dma_start(
                            out=out_re[:, 4 * mc + j, 0:D], in_=fm_sb[:, j, :]
                        )

    nc.finalize()
    return nc


def kernel(features, text, W, b):
    features = np.ascontiguousarray(features, dtype=np.float32)
    text = np.ascontiguousarray(text, dtype=np.float32)
    W = np.ascontiguousarray(W, dtype=np.float32)
    b = np.ascontiguousarray(b, dtype=np.float32)

    if "nc" not in _NC_CACHE:
        _NC_CACHE["nc"] = build_nc()
    nc = _NC_CACHE["nc"]

    t1 = np.ascontiguousarray(text[1])
    in_maps = [
        {"features": np.ascontiguousarray(features[i]), "t1": t1, "W": W, "b": b}
        for i in range(B)
    ]
    res = run_bass_kernel_spmd(nc, in_maps, core_ids=list(range(B)))
    return np.stack([res.results[i]["out"] for i in range(B)], axis=0)


if __name__ == "__main__":
    rng = np.random.default_rng(0)
    inputs = {
        "features": rng.standard_normal((B, N, D)).astype(np.float32),
        "text": rng.standard_normal((2, M, D)).astype(np.float32),
        "W": (rng.standard_normal((D, D)) * 0.02).astype(np.float32),
        "b": (rng.standard_normal((D,)) * 0.02).astype(np.float32),
    }
    out = kernel(**inputs)
    print("out", out.shape, out.dtype)

